# revision 16
# baseline (speedup 1.0000x reference)
"""Trainium2 Bass kernel for nn_Aggregation_Attn.

Computation (per sample i):
    scores[l] = sum_{t,c} q[i,t,c,l] * k[t,i,c]        # contraction over tc=t*c
    s         = softmax(scores)                         # over l
    out[t,c]  = sum_l q[i,t,c,l] * s[l]                 # contraction over l

Shapes: q [32, 64, 256, 64] f32, k [64, 32, 256] f32, out [64, 32, 256] f32.
Data-parallel over n across 8 cores (4 samples/core).

Per-core layout: q sample flattened to [tc=16384, l=64] (contiguous in HBM),
loaded into SBUF as [128 partitions, 8192] (partition p holds tc rows
[p*128, (p+1)*128), each partition row is one contiguous 32 KB HBM chunk).

Phase 1 on TensorE: for each tc0 in 0..127, matmul with stationary k column
k_sb[:, tc0] ([128,1]) and moving q slice [128, 64] -> accumulate psum [1, 64].
Softmax on tiny [1, 64] (DVE reduce_max -> ACT exp with accum -> DVE recip).
Phase 2 on VectorE: q *= s (broadcast over tc), then reduce over l.
"""

import numpy as np

import concourse.bacc as bacc
import concourse.bass as bass
import concourse.mybir as mybir
import concourse.tile as tile
from concourse import bass_utils

N, T, C, L = 32, 64, 256, 64
NCORES = 8
NS = N // NCORES  # samples per core
P = 128

_NC_CACHE = {}

VARIANT = "fp16"  # "fp32" | "fp16"


def build_nc(ns=NS, t=T, c=C, l=L, variant=None, repeat=1):
    variant = VARIANT if variant is None else variant
    key = (ns, t, c, l, variant, repeat)
    if key in _NC_CACHE:
        return _NC_CACHE[key]

    f32 = mybir.dt.float32
    tc_sz = t * c
    F = tc_sz // P  # tc rows per partition
    assert tc_sz % P == 0
    # k/out per-partition regrouping: partition p holds flat tc in
    # [p*F, (p+1)*F); requires c % F == 0 or F % c == 0.
    nc = bacc.Bacc("TRN2", target_bir_lowering=False, debug=False)
    q_d = nc.dram_tensor("q", [ns, t, c, l], f32, kind="ExternalInput")
    k_d = nc.dram_tensor("k", [t, ns, c], f32, kind="ExternalInput")
    o_d = nc.dram_tensor("o", [t, ns, c], f32, kind="ExternalOutput")

    body = {"fp32": _body, "fp16": _body_fp16}[variant]
    with tile.TileContext(nc) as tc_ctx:
        if repeat == 1:
            body(tc_ctx, q_d, k_d, o_d, ns, t, c, l)
        else:
            with tc_ctx.For_i(0, repeat, 1):
                body(tc_ctx, q_d, k_d, o_d, ns, t, c, l)
    nc.compile()
    _NC_CACHE[key] = nc
    return nc


def _body_fp16(tc_ctx, q_d, k_d, o_d, ns, t, c, l):
    """fp16 compute path: q/k cast to fp16 during the SWDGE load; fp16
    matmuls for scores; fp16 multiply + pairwise-tree reduce for phase 2.
    Softmax itself stays fp32 (PSUM accumulation is fp32)."""
    from contextlib import ExitStack

    nc = tc_ctx.nc
    f32 = mybir.dt.float32
    f16 = mybir.dt.float16
    tc_sz = t * c
    F = tc_sz // P

    stack = ExitStack()
    qpool = stack.enter_context(tc_ctx.tile_pool(name="qpool", bufs=3))
    kpool = stack.enter_context(tc_ctx.tile_pool(name="kpool", bufs=2))
    small = stack.enter_context(tc_ctx.tile_pool(name="small", bufs=4))
    opool = stack.enter_context(tc_ctx.tile_pool(name="opool", bufs=2))
    pscore = stack.enter_context(tc_ctx.tile_pool(name="pscore", bufs=2, space="PSUM"))

    G = 4  # q load chunks per sample (phase-1 overlaps the load)
    Fc = F // G

    for i in range(ns):
        # ---- loads (SWDGE casts f32 -> fp16 in the DMA), chunked ----
        k_t = kpool.tile([P, F], f16)
        nc.gpsimd.dma_start(
            out=k_t[:], in_=_flat_sample_kc(k_d.ap()[:, i, :], t, c, F)
        )

        q_src = q_d.ap()[i].rearrange("t c l -> (t c l)").rearrange(
            "(p x) -> p x", p=P
        )
        q_t = qpool.tile([P, F * l], f16)
        for g in range(G):
            nc.gpsimd.dma_start(
                out=q_t[:, g * Fc * l : (g + 1) * Fc * l],
                in_=q_src[:, g * Fc * l : (g + 1) * Fc * l],
            )

        # ---- phase 1: scores[l] = sum_tc q*k  (PE fp16, PSUM fp32) ----
        q3 = q_t[:].rearrange("p (f l) -> p f l", l=l)
        ps = pscore.tile([1, l], f32)
        for f in range(F):
            nc.tensor.matmul(
                ps[:],
                lhsT=k_t[:, f : f + 1],
                rhs=q3[:, f, :],
                start=(f == 0),
                stop=(f == F - 1),
            )

        # ---- softmax on [1, l] (fp32) ----
        negmax = small.tile([1, 1], f32)
        nc.vector.tensor_reduce(
            out=negmax[:], in_=ps[:], axis=mybir.AxisListType.X,
            op=mybir.AluOpType.max, negate=True,
        )
        exps16 = small.tile([1, l], f16)
        sumexp = small.tile([1, 1], f32)
        nc.scalar.activation(
            out=exps16[:], in_=ps[:], func=mybir.ActivationFunctionType.Exp,
            bias=negmax[:], scale=1.0, accum_out=sumexp[:],
        )
        # Deferred normalization: multiply by exp now, scale by 1/sum at the
        # end (keeps the reciprocal off the critical path).
        rsum = small.tile([1, 1], f32)
        nc.vector.reciprocal(out=rsum[:], in_=sumexp[:])
        rrep = small.tile([P, 1], f32)
        nc.gpsimd.partition_broadcast(rrep[:], rsum[:])

        # broadcast exp to all partitions
        srep = small.tile([P, l], f16)
        nc.gpsimd.partition_broadcast(srep[:], exps16[:])

        # ---- phase 2: q *= s, then pairwise tree-sum over l ----
        s_b = srep[:].unsqueeze(1).to_broadcast([P, F, l])
        nc.vector.tensor_tensor(out=q3, in0=q3, in1=s_b, op=mybir.AluOpType.mult)
        hh = l // 2
        while hh >= 2:
            nc.vector.tensor_tensor(
                out=q3[:, :, 0:hh],
                in0=q3[:, :, 0:hh],
                in1=q3[:, :, hh : 2 * hh],
                op=mybir.AluOpType.add,
            )
            hh //= 2
        ored = opool.tile([P, F], f32)
        nc.vector.tensor_tensor(
            out=ored[:],
            in0=q3[:, :, 0],
            in1=q3[:, :, 1],
            op=mybir.AluOpType.add,
        )
        # deferred softmax normalization
        nc.vector.tensor_scalar_mul(out=ored[:], in0=ored[:], scalar1=rrep[:])

        # ---- store ----
        nc.sync.dma_start(
            out=_flat_sample_kc(o_d.ap()[:, i, :], t, c, F), in_=ored[:]
        )

    stack.close()


def _flat_sample_kc(ap2d, t, c, F):
    """[t, c] AP -> AP iterating flat tc grouped as P partitions x F.

    Returned AP may be 3-dim; DMA matches flat element order, so it pairs
    with a [P, F] SBUF tile.
    """
    if c % F == 0:
        hh = c // F
        return ap2d.rearrange("t (hh f) -> t hh f", hh=hh)
    else:
        assert F % c == 0
        g = F // c  # whole t-rows per partition
        return ap2d.rearrange("(p g) c -> p (g c)", g=g)


def _body(tc_ctx, q_d, k_d, o_d, ns, t, c, l):
    from contextlib import ExitStack

    nc = tc_ctx.nc
    f32 = mybir.dt.float32
    tc_sz = t * c
    F = tc_sz // P

    stack = ExitStack()
    qpool = stack.enter_context(tc_ctx.tile_pool(name="qpool", bufs=3))
    kpool = stack.enter_context(tc_ctx.tile_pool(name="kpool", bufs=2))
    small = stack.enter_context(tc_ctx.tile_pool(name="small", bufs=4))
    opool = stack.enter_context(tc_ctx.tile_pool(name="opool", bufs=2))
    pscore = stack.enter_context(tc_ctx.tile_pool(name="pscore", bufs=2, space="PSUM"))

    for i in range(ns):
        # ---- loads ----
        q_t = qpool.tile([P, F * l], f32)
        q_src = q_d.ap()[i].rearrange("t c l -> (t c l)").rearrange(
            "(p x) -> p x", p=P
        )
        nc.sync.dma_start(out=q_t[:], in_=q_src)

        k_t = kpool.tile([P, F], f32)
        nc.sync.dma_start(out=k_t[:], in_=_flat_sample_kc(k_d.ap()[:, i, :], t, c, F))

        # ---- phase 1: scores[l] = sum_tc q*k  (PE, PSUM accumulation) ----
        q3 = q_t[:].rearrange("p (f l) -> p f l", l=l)
        ps = pscore.tile([1, l], f32)
        for f in range(F):
            nc.tensor.matmul(
                ps[:],
                lhsT=k_t[:, f : f + 1],
                rhs=q3[:, f, :],
                start=(f == 0),
                stop=(f == F - 1),
            )

        # ---- softmax on [1, l] ----
        negmax = small.tile([1, 1], f32)
        nc.vector.tensor_reduce(
            out=negmax[:], in_=ps[:], axis=mybir.AxisListType.X,
            op=mybir.AluOpType.max, negate=True,
        )
        exps = small.tile([1, l], f32)
        sumexp = small.tile([1, 1], f32)
        nc.scalar.activation(
            out=exps[:], in_=ps[:], func=mybir.ActivationFunctionType.Exp,
            bias=negmax[:], scale=1.0, accum_out=sumexp[:],
        )
        rsum = small.tile([1, 1], f32)
        nc.vector.reciprocal(out=rsum[:], in_=sumexp[:])
        srow = small.tile([1, l], f32)
        nc.vector.tensor_scalar_mul(out=srow[:], in0=exps[:], scalar1=rsum[:])

        # broadcast s to all partitions
        srep = small.tile([P, l], f32)
        nc.gpsimd.partition_broadcast(srep[:], srow[:])

        # ---- phase 2: q *= s (broadcast over f) ; reduce over l ----
        s_b = srep[:].unsqueeze(1).to_broadcast([P, F, l])
        nc.vector.tensor_tensor(
            out=q3, in0=q3, in1=s_b, op=mybir.AluOpType.mult
        )
        ored = opool.tile([P, F], f32)
        nc.vector.tensor_reduce(
            out=ored[:], in_=q3, axis=mybir.AxisListType.X,
            op=mybir.AluOpType.add,
        )

        # ---- store ----
        nc.sync.dma_start(
            out=_flat_sample_kc(o_d.ap()[:, i, :], t, c, F), in_=ored[:]
        )

    stack.close()


def run(query, key, repeat=1, variant=None, **spmd_kwargs):
    query = np.ascontiguousarray(np.asarray(query, dtype=np.float32))
    key = np.asarray(key, dtype=np.float32)
    n, t, c, l = query.shape
    ncores = NCORES
    ns = n // ncores
    nc = build_nc(ns, t, c, l, variant=variant, repeat=repeat)

    in_maps = []
    for i in range(ncores):
        in_maps.append(
            {
                "q": np.ascontiguousarray(query[i * ns : (i + 1) * ns]),
                "k": np.ascontiguousarray(key[:, i * ns : (i + 1) * ns, :]),
            }
        )
    res = bass_utils.run_bass_kernel_spmd(
        nc, in_maps, core_ids=list(range(ncores)), **spmd_kwargs
    )
    out = np.empty((t, n, c), dtype=np.float32)
    for i in range(ncores):
        out[:, i * ns : (i + 1) * ns, :] = res.results[i]["o"]
    return out, res


def kernel(**inputs):
    out, _ = run(inputs["query"], inputs["key"])
    return out


# revision 17
# speedup vs baseline: 1.6924x; 1.6924x over previous
"""Trainium2 Bass kernel for nn_Aggregation_Attn.

Computation (per sample i):
    scores[l] = sum_{t,c} q[i,t,c,l] * k[t,i,c]        # contraction over tc=t*c
    s         = softmax(scores)                         # over l
    out[t,c]  = sum_l q[i,t,c,l] * s[l]                 # contraction over l

Shapes: q [32, 64, 256, 64] f32, k [64, 32, 256] f32, out [64, 32, 256] f32.
Data-parallel over n across 8 cores (4 samples/core).

Per-core layout: q sample flattened to [tc=16384, l=64] (contiguous in HBM),
loaded into SBUF as [128 partitions, 8192] (partition p holds tc rows
[p*128, (p+1)*128), each partition row is one contiguous 32 KB HBM chunk).

Phase 1 on TensorE: for each tc0 in 0..127, matmul with stationary k column
k_sb[:, tc0] ([128,1]) and moving q slice [128, 64] -> accumulate psum [1, 64].
Softmax on tiny [1, 64] (DVE reduce_max -> ACT exp with accum -> DVE recip).
Phase 2 on VectorE: q *= s (broadcast over tc), then reduce over l.
"""

import numpy as np

import concourse.bacc as bacc
import concourse.bass as bass
import concourse.mybir as mybir
import concourse.tile as tile
from concourse import bass_utils

N, T, C, L = 32, 64, 256, 64
NCORES = 8
NS = N // NCORES  # samples per core
P = 128

_NC_CACHE = {}

VARIANT = "fp16"  # "fp32" | "fp16"


def build_nc(ns=NS, t=T, c=C, l=L, variant=None, repeat=1):
    variant = VARIANT if variant is None else variant
    key = (ns, t, c, l, variant, repeat)
    if key in _NC_CACHE:
        return _NC_CACHE[key]

    f32 = mybir.dt.float32
    tc_sz = t * c
    F = tc_sz // P  # tc rows per partition
    assert tc_sz % P == 0
    # k/out per-partition regrouping: partition p holds flat tc in
    # [p*F, (p+1)*F); requires c % F == 0 or F % c == 0.
    nc = bacc.Bacc("TRN2", target_bir_lowering=False, debug=False)
    q_d = nc.dram_tensor("q", [ns, t, c, l], f32, kind="ExternalInput")
    k_d = nc.dram_tensor("k", [t, ns, c], f32, kind="ExternalInput")
    o_d = nc.dram_tensor("o", [t, ns, c], f32, kind="ExternalOutput")

    body = {"fp32": _body, "fp16": _body_fp16}[variant]
    with tile.TileContext(nc) as tc_ctx:
        if repeat == 1:
            body(tc_ctx, q_d, k_d, o_d, ns, t, c, l)
        else:
            # PE body far exceeds one IRAM block; hint the back-edge prefetch.
            with tc_ctx.For_i(
                0, repeat, 1, hint_engines=(mybir.EngineType.PE,)
            ):
                body(tc_ctx, q_d, k_d, o_d, ns, t, c, l)
    nc.compile()
    _NC_CACHE[key] = nc
    return nc


def _body_fp16(tc_ctx, q_d, k_d, o_d, ns, t, c, l):
    """fp16 compute path: q/k cast to fp16 during the SWDGE load; fp16
    matmuls for scores; fp16 multiply + pairwise-tree reduce for phase 2.
    Softmax itself stays fp32 (PSUM accumulation is fp32)."""
    from contextlib import ExitStack

    nc = tc_ctx.nc
    f32 = mybir.dt.float32
    f16 = mybir.dt.float16
    tc_sz = t * c
    F = tc_sz // P

    stack = ExitStack()
    qpool = stack.enter_context(tc_ctx.tile_pool(name="qpool", bufs=3))
    kpool = stack.enter_context(tc_ctx.tile_pool(name="kpool", bufs=2))
    small = stack.enter_context(tc_ctx.tile_pool(name="small", bufs=4))
    opool = stack.enter_context(tc_ctx.tile_pool(name="opool", bufs=2))
    pscore = stack.enter_context(tc_ctx.tile_pool(name="pscore", bufs=2, space="PSUM"))

    G = 4  # q load chunks per sample (phase-1 overlaps the load)
    Fc = F // G

    for i in range(ns):
        # ---- loads (SWDGE casts f32 -> fp16 in the DMA), chunked ----
        k_t = kpool.tile([P, F], f16)
        nc.gpsimd.dma_start(
            out=k_t[:], in_=_flat_sample_kc(k_d.ap()[:, i, :], t, c, F)
        )

        q_src = q_d.ap()[i].rearrange("t c l -> (t c l)").rearrange(
            "(p x) -> p x", p=P
        )
        q_t = qpool.tile([P, F * l], f16)
        for g in range(G):
            nc.gpsimd.dma_start(
                out=q_t[:, g * Fc * l : (g + 1) * Fc * l],
                in_=q_src[:, g * Fc * l : (g + 1) * Fc * l],
            )

        # ---- phase 1: scores[l] = sum_tc q*k  (PE fp16, PSUM fp32) ----
        q3 = q_t[:].rearrange("p (f l) -> p f l", l=l)
        ps = pscore.tile([1, l], f32)
        for f in range(F):
            nc.tensor.matmul(
                ps[:],
                lhsT=k_t[:, f : f + 1],
                rhs=q3[:, f, :],
                start=(f == 0),
                stop=(f == F - 1),
            )

        # ---- softmax on [1, l] (fp32) ----
        negmax = small.tile([1, 1], f32)
        nc.vector.tensor_reduce(
            out=negmax[:], in_=ps[:], axis=mybir.AxisListType.X,
            op=mybir.AluOpType.max, negate=True,
        )
        exps16 = small.tile([1, l], f16)
        sumexp = small.tile([1, 1], f32)
        nc.scalar.activation(
            out=exps16[:], in_=ps[:], func=mybir.ActivationFunctionType.Exp,
            bias=negmax[:], scale=1.0, accum_out=sumexp[:],
        )
        # Deferred normalization: multiply by exp now, scale by 1/sum at the
        # end (keeps the reciprocal off the critical path).
        rsum = small.tile([1, 1], f32)
        nc.vector.reciprocal(out=rsum[:], in_=sumexp[:])
        rrep = small.tile([P, 1], f32)
        nc.gpsimd.partition_broadcast(rrep[:], rsum[:])

        # broadcast exp to all partitions
        srep = small.tile([P, l], f16)
        nc.gpsimd.partition_broadcast(srep[:], exps16[:])

        # ---- phase 2: q *= s, then pairwise tree-sum over l ----
        s_b = srep[:].unsqueeze(1).to_broadcast([P, F, l])
        nc.vector.tensor_tensor(out=q3, in0=q3, in1=s_b, op=mybir.AluOpType.mult)
        hh = l // 2
        while hh >= 2:
            nc.vector.tensor_tensor(
                out=q3[:, :, 0:hh],
                in0=q3[:, :, 0:hh],
                in1=q3[:, :, hh : 2 * hh],
                op=mybir.AluOpType.add,
            )
            hh //= 2
        ored = opool.tile([P, F], f32)
        nc.vector.tensor_tensor(
            out=ored[:],
            in0=q3[:, :, 0],
            in1=q3[:, :, 1],
            op=mybir.AluOpType.add,
        )
        # deferred softmax normalization
        nc.vector.tensor_scalar_mul(out=ored[:], in0=ored[:], scalar1=rrep[:])

        # ---- store ----
        nc.sync.dma_start(
            out=_flat_sample_kc(o_d.ap()[:, i, :], t, c, F), in_=ored[:]
        )

    stack.close()


def _flat_sample_kc(ap2d, t, c, F):
    """[t, c] AP -> AP iterating flat tc grouped as P partitions x F.

    Returned AP may be 3-dim; DMA matches flat element order, so it pairs
    with a [P, F] SBUF tile.
    """
    if c % F == 0:
        hh = c // F
        return ap2d.rearrange("t (hh f) -> t hh f", hh=hh)
    else:
        assert F % c == 0
        g = F // c  # whole t-rows per partition
        return ap2d.rearrange("(p g) c -> p (g c)", g=g)


def _body(tc_ctx, q_d, k_d, o_d, ns, t, c, l):
    from contextlib import ExitStack

    nc = tc_ctx.nc
    f32 = mybir.dt.float32
    tc_sz = t * c
    F = tc_sz // P

    stack = ExitStack()
    qpool = stack.enter_context(tc_ctx.tile_pool(name="qpool", bufs=3))
    kpool = stack.enter_context(tc_ctx.tile_pool(name="kpool", bufs=2))
    small = stack.enter_context(tc_ctx.tile_pool(name="small", bufs=4))
    opool = stack.enter_context(tc_ctx.tile_pool(name="opool", bufs=2))
    pscore = stack.enter_context(tc_ctx.tile_pool(name="pscore", bufs=2, space="PSUM"))

    for i in range(ns):
        # ---- loads ----
        q_t = qpool.tile([P, F * l], f32)
        q_src = q_d.ap()[i].rearrange("t c l -> (t c l)").rearrange(
            "(p x) -> p x", p=P
        )
        nc.sync.dma_start(out=q_t[:], in_=q_src)

        k_t = kpool.tile([P, F], f32)
        nc.sync.dma_start(out=k_t[:], in_=_flat_sample_kc(k_d.ap()[:, i, :], t, c, F))

        # ---- phase 1: scores[l] = sum_tc q*k  (PE, PSUM accumulation) ----
        q3 = q_t[:].rearrange("p (f l) -> p f l", l=l)
        ps = pscore.tile([1, l], f32)
        for f in range(F):
            nc.tensor.matmul(
                ps[:],
                lhsT=k_t[:, f : f + 1],
                rhs=q3[:, f, :],
                start=(f == 0),
                stop=(f == F - 1),
            )

        # ---- softmax on [1, l] ----
        negmax = small.tile([1, 1], f32)
        nc.vector.tensor_reduce(
            out=negmax[:], in_=ps[:], axis=mybir.AxisListType.X,
            op=mybir.AluOpType.max, negate=True,
        )
        exps = small.tile([1, l], f32)
        sumexp = small.tile([1, 1], f32)
        nc.scalar.activation(
            out=exps[:], in_=ps[:], func=mybir.ActivationFunctionType.Exp,
            bias=negmax[:], scale=1.0, accum_out=sumexp[:],
        )
        rsum = small.tile([1, 1], f32)
        nc.vector.reciprocal(out=rsum[:], in_=sumexp[:])
        srow = small.tile([1, l], f32)
        nc.vector.tensor_scalar_mul(out=srow[:], in0=exps[:], scalar1=rsum[:])

        # broadcast s to all partitions
        srep = small.tile([P, l], f32)
        nc.gpsimd.partition_broadcast(srep[:], srow[:])

        # ---- phase 2: q *= s (broadcast over f) ; reduce over l ----
        s_b = srep[:].unsqueeze(1).to_broadcast([P, F, l])
        nc.vector.tensor_tensor(
            out=q3, in0=q3, in1=s_b, op=mybir.AluOpType.mult
        )
        ored = opool.tile([P, F], f32)
        nc.vector.tensor_reduce(
            out=ored[:], in_=q3, axis=mybir.AxisListType.X,
            op=mybir.AluOpType.add,
        )

        # ---- store ----
        nc.sync.dma_start(
            out=_flat_sample_kc(o_d.ap()[:, i, :], t, c, F), in_=ored[:]
        )

    stack.close()


def run(query, key, repeat=1, variant=None, **spmd_kwargs):
    query = np.ascontiguousarray(np.asarray(query, dtype=np.float32))
    key = np.asarray(key, dtype=np.float32)
    n, t, c, l = query.shape
    ncores = NCORES
    ns = n // ncores
    nc = build_nc(ns, t, c, l, variant=variant, repeat=repeat)

    in_maps = []
    for i in range(ncores):
        in_maps.append(
            {
                "q": np.ascontiguousarray(query[i * ns : (i + 1) * ns]),
                "k": np.ascontiguousarray(key[:, i * ns : (i + 1) * ns, :]),
            }
        )
    res = bass_utils.run_bass_kernel_spmd(
        nc, in_maps, core_ids=list(range(ncores)), **spmd_kwargs
    )
    out = np.empty((t, n, c), dtype=np.float32)
    for i in range(ncores):
        out[:, i * ns : (i + 1) * ns, :] = res.results[i]["o"]
    return out, res


def kernel(**inputs):
    out, _ = run(inputs["query"], inputs["key"])
    return out


# revision 34
# speedup vs baseline: 1.9771x; 1.1682x over previous
"""Trainium2 Bass kernel for nn_Aggregation_Attn.

Computation (per sample i):
    scores[l] = sum_{t,c} q[i,t,c,l] * k[t,i,c]        # contraction over tc=t*c
    s         = softmax(scores)                         # over l
    out[t,c]  = sum_l q[i,t,c,l] * s[l]                 # contraction over l

Shapes: q [32, 64, 256, 64] f32, k [64, 32, 256] f32, out [64, 32, 256] f32.
Data-parallel over n across 8 cores (4 samples/core).

Per-core layout: q sample flattened to [tc=16384, l=64] (contiguous in HBM),
loaded into SBUF as [128 partitions, 8192] (partition p holds tc rows
[p*128, (p+1)*128), each partition row is one contiguous 32 KB HBM chunk).

Phase 1 on TensorE: for each tc0 in 0..127, matmul with stationary k column
k_sb[:, tc0] ([128,1]) and moving q slice [128, 64] -> accumulate psum [1, 64].
Softmax on tiny [1, 64] (DVE reduce_max -> ACT exp with accum -> DVE recip).
Phase 2 on VectorE: q *= s (broadcast over tc), then reduce over l.
"""

import numpy as np

import concourse.bacc as bacc
import concourse.bass as bass
import concourse.mybir as mybir
import concourse.tile as tile
from concourse import bass_utils

N, T, C, L = 32, 64, 256, 64
NCORES = 8
NS = N // NCORES  # samples per core
P = 128

_NC_CACHE = {}

VARIANT = "fp16hw"
G_LOADS = 8  # q load chunks per sample
DUAL_RING = False  # alternate q-chunk DMAs between SP and ACT HWDGE rings


def build_nc(ns=NS, t=T, c=C, l=L, variant=None, repeat=1):
    variant = VARIANT if variant is None else variant
    key = (ns, t, c, l, variant, repeat, G_LOADS, DUAL_RING)
    if key in _NC_CACHE:
        return _NC_CACHE[key]

    f32 = mybir.dt.float32
    tc_sz = t * c
    F = tc_sz // P  # tc rows per partition
    assert tc_sz % P == 0
    # k/out per-partition regrouping: partition p holds flat tc in
    # [p*F, (p+1)*F); requires c % F == 0 or F % c == 0.
    nc = bacc.Bacc("TRN2", target_bir_lowering=False, debug=False)
    q_d = nc.dram_tensor("q", [ns, t, c, l], f32, kind="ExternalInput")
    k_d = nc.dram_tensor("k", [t, ns, c], f32, kind="ExternalInput")
    o_d = nc.dram_tensor("o", [t, ns, c], f32, kind="ExternalOutput")

    body = {
        "fp32": _body,
        "fp16": _body_fp16,
        "fp16hw": _body_fp16hw,
        "dmaonly": _body_dmaonly,
        "dmahw": _body_dmahw,
    }[variant]
    with tile.TileContext(nc) as tc_ctx:
        if repeat == 1:
            body(tc_ctx, q_d, k_d, o_d, ns, t, c, l)
        else:
            # PE body far exceeds one IRAM block; hint the back-edge prefetch.
            with tc_ctx.For_i(
                0, repeat, 1, hint_engines=(mybir.EngineType.PE,)
            ):
                body(tc_ctx, q_d, k_d, o_d, ns, t, c, l)
    nc.compile()
    _NC_CACHE[key] = nc
    return nc


def _body_fp16(tc_ctx, q_d, k_d, o_d, ns, t, c, l):
    """fp16 compute path: q/k cast to fp16 during the SWDGE load; fp16
    matmuls for scores; fp16 multiply + pairwise-tree reduce for phase 2.
    Softmax itself stays fp32 (PSUM accumulation is fp32)."""
    from contextlib import ExitStack

    nc = tc_ctx.nc
    f32 = mybir.dt.float32
    f16 = mybir.dt.float16
    tc_sz = t * c
    F = tc_sz // P

    stack = ExitStack()
    qpool = stack.enter_context(tc_ctx.tile_pool(name="qpool", bufs=3))
    kpool = stack.enter_context(tc_ctx.tile_pool(name="kpool", bufs=2))
    small = stack.enter_context(tc_ctx.tile_pool(name="small", bufs=4))
    opool = stack.enter_context(tc_ctx.tile_pool(name="opool", bufs=2))
    pscore = stack.enter_context(tc_ctx.tile_pool(name="pscore", bufs=2, space="PSUM"))

    G = 4  # q load chunks per sample (phase-1 overlaps the load)
    Fc = F // G

    for i in range(ns):
        # ---- loads (SWDGE casts f32 -> fp16 in the DMA), chunked ----
        k_t = kpool.tile([P, F], f16)
        nc.gpsimd.dma_start(
            out=k_t[:], in_=_flat_sample_kc(k_d.ap()[:, i, :], t, c, F)
        )

        q_src = q_d.ap()[i].rearrange("t c l -> (t c l)").rearrange(
            "(p x) -> p x", p=P
        )
        q_t = qpool.tile([P, F * l], f16)
        for g in range(G):
            nc.gpsimd.dma_start(
                out=q_t[:, g * Fc * l : (g + 1) * Fc * l],
                in_=q_src[:, g * Fc * l : (g + 1) * Fc * l],
            )

        # ---- phase 1: scores[l] = sum_tc q*k  (PE fp16, PSUM fp32) ----
        q3 = q_t[:].rearrange("p (f l) -> p f l", l=l)
        ps = pscore.tile([1, l], f32)
        for f in range(F):
            nc.tensor.matmul(
                ps[:],
                lhsT=k_t[:, f : f + 1],
                rhs=q3[:, f, :],
                start=(f == 0),
                stop=(f == F - 1),
            )

        # ---- softmax on [1, l] (fp32) ----
        negmax = small.tile([1, 1], f32)
        nc.vector.tensor_reduce(
            out=negmax[:], in_=ps[:], axis=mybir.AxisListType.X,
            op=mybir.AluOpType.max, negate=True,
        )
        exps16 = small.tile([1, l], f16)
        sumexp = small.tile([1, 1], f32)
        nc.scalar.activation(
            out=exps16[:], in_=ps[:], func=mybir.ActivationFunctionType.Exp,
            bias=negmax[:], scale=1.0, accum_out=sumexp[:],
        )
        # Deferred normalization: multiply by exp now, scale by 1/sum at the
        # end (keeps the reciprocal off the critical path).
        rsum = small.tile([1, 1], f32)
        nc.vector.reciprocal(out=rsum[:], in_=sumexp[:])
        rrep = small.tile([P, 1], f32)
        nc.gpsimd.partition_broadcast(rrep[:], rsum[:])

        # broadcast exp to all partitions
        srep = small.tile([P, l], f16)
        nc.gpsimd.partition_broadcast(srep[:], exps16[:])

        # ---- phase 2: q *= s, then pairwise tree-sum over l ----
        s_b = srep[:].unsqueeze(1).to_broadcast([P, F, l])
        nc.vector.tensor_tensor(out=q3, in0=q3, in1=s_b, op=mybir.AluOpType.mult)
        hh = l // 2
        while hh >= 2:
            nc.vector.tensor_tensor(
                out=q3[:, :, 0:hh],
                in0=q3[:, :, 0:hh],
                in1=q3[:, :, hh : 2 * hh],
                op=mybir.AluOpType.add,
            )
            hh //= 2
        ored = opool.tile([P, F], f32)
        nc.vector.tensor_tensor(
            out=ored[:],
            in0=q3[:, :, 0],
            in1=q3[:, :, 1],
            op=mybir.AluOpType.add,
        )
        # deferred softmax normalization
        nc.vector.tensor_scalar_mul(out=ored[:], in0=ored[:], scalar1=rrep[:])

        # ---- store ----
        nc.sync.dma_start(
            out=_flat_sample_kc(o_d.ap()[:, i, :], t, c, F), in_=ored[:]
        )

    stack.close()


def _flat_sample_kc(ap2d, t, c, F):
    """[t, c] AP -> AP iterating flat tc grouped as P partitions x F.

    Returned AP may be 3-dim; DMA matches flat element order, so it pairs
    with a [P, F] SBUF tile.
    """
    if c % F == 0:
        hh = c // F
        return ap2d.rearrange("t (hh f) -> t hh f", hh=hh)
    else:
        assert F % c == 0
        g = F // c  # whole t-rows per partition
        return ap2d.rearrange("(p g) c -> p (g c)", g=g)


def _body(tc_ctx, q_d, k_d, o_d, ns, t, c, l):
    from contextlib import ExitStack

    nc = tc_ctx.nc
    f32 = mybir.dt.float32
    tc_sz = t * c
    F = tc_sz // P

    stack = ExitStack()
    qpool = stack.enter_context(tc_ctx.tile_pool(name="qpool", bufs=3))
    kpool = stack.enter_context(tc_ctx.tile_pool(name="kpool", bufs=2))
    small = stack.enter_context(tc_ctx.tile_pool(name="small", bufs=4))
    opool = stack.enter_context(tc_ctx.tile_pool(name="opool", bufs=2))
    pscore = stack.enter_context(tc_ctx.tile_pool(name="pscore", bufs=2, space="PSUM"))

    for i in range(ns):
        # ---- loads ----
        q_t = qpool.tile([P, F * l], f32)
        q_src = q_d.ap()[i].rearrange("t c l -> (t c l)").rearrange(
            "(p x) -> p x", p=P
        )
        nc.sync.dma_start(out=q_t[:], in_=q_src)

        k_t = kpool.tile([P, F], f32)
        nc.sync.dma_start(out=k_t[:], in_=_flat_sample_kc(k_d.ap()[:, i, :], t, c, F))

        # ---- phase 1: scores[l] = sum_tc q*k  (PE, PSUM accumulation) ----
        q3 = q_t[:].rearrange("p (f l) -> p f l", l=l)
        ps = pscore.tile([1, l], f32)
        for f in range(F):
            nc.tensor.matmul(
                ps[:],
                lhsT=k_t[:, f : f + 1],
                rhs=q3[:, f, :],
                start=(f == 0),
                stop=(f == F - 1),
            )

        # ---- softmax on [1, l] ----
        negmax = small.tile([1, 1], f32)
        nc.vector.tensor_reduce(
            out=negmax[:], in_=ps[:], axis=mybir.AxisListType.X,
            op=mybir.AluOpType.max, negate=True,
        )
        exps = small.tile([1, l], f32)
        sumexp = small.tile([1, 1], f32)
        nc.scalar.activation(
            out=exps[:], in_=ps[:], func=mybir.ActivationFunctionType.Exp,
            bias=negmax[:], scale=1.0, accum_out=sumexp[:],
        )
        rsum = small.tile([1, 1], f32)
        nc.vector.reciprocal(out=rsum[:], in_=sumexp[:])
        srow = small.tile([1, l], f32)
        nc.vector.tensor_scalar_mul(out=srow[:], in0=exps[:], scalar1=rsum[:])

        # broadcast s to all partitions
        srep = small.tile([P, l], f32)
        nc.gpsimd.partition_broadcast(srep[:], srow[:])

        # ---- phase 2: q *= s (broadcast over f) ; reduce over l ----
        s_b = srep[:].unsqueeze(1).to_broadcast([P, F, l])
        nc.vector.tensor_tensor(
            out=q3, in0=q3, in1=s_b, op=mybir.AluOpType.mult
        )
        ored = opool.tile([P, F], f32)
        nc.vector.tensor_reduce(
            out=ored[:], in_=q3, axis=mybir.AxisListType.X,
            op=mybir.AluOpType.add,
        )

        # ---- store ----
        nc.sync.dma_start(
            out=_flat_sample_kc(o_d.ap()[:, i, :], t, c, F), in_=ored[:]
        )

    stack.close()


def _body_dmaonly(tc_ctx, q_d, k_d, o_d, ns, t, c, l):
    """Timing probe: SWDGE cast loads only, no compute."""
    from contextlib import ExitStack

    nc = tc_ctx.nc
    f32 = mybir.dt.float32
    f16 = mybir.dt.float16
    F = (t * c) // P
    stack = ExitStack()
    qpool = stack.enter_context(tc_ctx.tile_pool(name="qpool", bufs=3))
    kpool = stack.enter_context(tc_ctx.tile_pool(name="kpool", bufs=2))
    opool = stack.enter_context(tc_ctx.tile_pool(name="opool", bufs=2))
    G = 4
    Fc = F // G
    for i in range(ns):
        k_t = kpool.tile([P, F], f16)
        nc.gpsimd.dma_start(
            out=k_t[:], in_=_flat_sample_kc(k_d.ap()[:, i, :], t, c, F)
        )
        q_src = q_d.ap()[i].rearrange("t c l -> (t c l)").rearrange(
            "(p x) -> p x", p=P
        )
        q_t = qpool.tile([P, F * l], f16)
        for g in range(G):
            nc.gpsimd.dma_start(
                out=q_t[:, g * Fc * l : (g + 1) * Fc * l],
                in_=q_src[:, g * Fc * l : (g + 1) * Fc * l],
            )
        ored = opool.tile([P, F], f32)
        # touch the loaded tile so the store depends on the loads
        nc.vector.tensor_copy(ored[:], q_t[:].rearrange("p (f l) -> p f l", l=l)[:, :, 0])
        nc.sync.dma_start(
            out=_flat_sample_kc(o_d.ap()[:, i, :], t, c, F), in_=ored[:]
        )
    stack.close()


def _body_dmahw(tc_ctx, q_d, k_d, o_d, ns, t, c, l):
    """Timing probe: HWDGE f32 loads only, no cast/compute."""
    from contextlib import ExitStack

    nc = tc_ctx.nc
    f32 = mybir.dt.float32
    F = (t * c) // P
    stack = ExitStack()
    q32pool = stack.enter_context(tc_ctx.tile_pool(name="q32pool", bufs=3))
    opool = stack.enter_context(tc_ctx.tile_pool(name="opool", bufs=2))
    G = G_LOADS
    Fc = F // G
    for i in range(ns):
        q_src = q_d.ap()[i].rearrange("t c l -> (t c l)").rearrange(
            "(p x) -> p x", p=P
        )
        q32 = q32pool.tile([P, F * l], f32)
        for g in range(G):
            sl = slice(g * Fc * l, (g + 1) * Fc * l)
            eng = nc.scalar if (DUAL_RING and g % 2) else nc.sync
            eng.dma_start(out=q32[:, sl], in_=q_src[:, sl])
        ored = opool.tile([P, F], f32)
        nc.vector.tensor_copy(
            ored[:], q32[:].rearrange("p (f l) -> p f l", l=l)[:, :, 0]
        )
        nc.scalar.dma_start(
            out=_flat_sample_kc(o_d.ap()[:, i, :], t, c, F), in_=ored[:]
        )
    stack.close()


def _body_fp16hw(tc_ctx, q_d, k_d, o_d, ns, t, c, l):
    """Like _body_fp16 but loads q as f32 via HWDGE (full DMA rate) and casts
    f32 -> fp16 on the (otherwise idle) ScalarE."""
    from contextlib import ExitStack

    nc = tc_ctx.nc
    f32 = mybir.dt.float32
    f16 = mybir.dt.float16
    F = (t * c) // P

    stack = ExitStack()
    q32pool = stack.enter_context(tc_ctx.tile_pool(name="q32pool", bufs=3))
    qpool = stack.enter_context(tc_ctx.tile_pool(name="qpool", bufs=2))
    kpool = stack.enter_context(tc_ctx.tile_pool(name="kpool", bufs=2))
    small = stack.enter_context(tc_ctx.tile_pool(name="small", bufs=4))
    opool = stack.enter_context(tc_ctx.tile_pool(name="opool", bufs=4))
    pscore = stack.enter_context(tc_ctx.tile_pool(name="pscore", bufs=2, space="PSUM"))

    G = G_LOADS
    Fc = F // G

    # All k loads upfront (SWDGE, tiny) so nothing later blocks them.
    k_ts = []
    for i in range(ns):
        k_t = kpool.tile([P, F], f16, tag=f"k{i}")
        nc.gpsimd.dma_start(
            out=k_t[:], in_=_flat_sample_kc(k_d.ap()[:, i, :], t, c, F)
        )
        k_ts.append(k_t)

    # Stores are emitted two samples late: a store's semaphore wait (on the
    # phase-2 result) must never block later q-load issues on the SP ring.
    pending_stores = []

    def flush_store():
        ap_out, tile_in = pending_stores.pop(0)
        nc.sync.dma_start(out=ap_out, in_=tile_in)

    for i in range(ns):
        k_t = k_ts[i]
        q_src = q_d.ap()[i].rearrange("t c l -> (t c l)").rearrange(
            "(p x) -> p x", p=P
        )
        q32 = q32pool.tile([P, F * l], f32)
        q_t = qpool.tile([P, F * l], f16)
        for g in range(G):
            sl = slice(g * Fc * l, (g + 1) * Fc * l)
            eng = nc.scalar if (DUAL_RING and g % 2) else nc.sync
            eng.dma_start(out=q32[:, sl], in_=q_src[:, sl])
            nc.scalar.copy(out=q_t[:, sl], in_=q32[:, sl])

        # ---- phase 1 ----
        q3 = q_t[:].rearrange("p (f l) -> p f l", l=l)
        ps = pscore.tile([1, l], f32)
        for f in range(F):
            nc.tensor.matmul(
                ps[:],
                lhsT=k_t[:, f : f + 1],
                rhs=q3[:, f, :],
                start=(f == 0),
                stop=(f == F - 1),
            )

        # ---- softmax (deferred normalization) ----
        negmax = small.tile([1, 1], f32)
        nc.vector.tensor_reduce(
            out=negmax[:], in_=ps[:], axis=mybir.AxisListType.X,
            op=mybir.AluOpType.max, negate=True,
        )
        exps16 = small.tile([1, l], f16)
        sumexp = small.tile([1, 1], f32)
        nc.scalar.activation(
            out=exps16[:], in_=ps[:], func=mybir.ActivationFunctionType.Exp,
            bias=negmax[:], scale=1.0, accum_out=sumexp[:],
        )
        # srep broadcast first (gates the phase-2 multiply); rrep is only
        # needed at the final scale, so it goes second on the POOL stream.
        srep = small.tile([P, l], f16)
        nc.gpsimd.partition_broadcast(srep[:], exps16[:])
        rsum = small.tile([1, 1], f32)
        nc.vector.reciprocal(out=rsum[:], in_=sumexp[:])
        rrep = small.tile([P, 1], f32)
        nc.gpsimd.partition_broadcast(rrep[:], rsum[:])

        # ---- phase 2 ----
        s_b = srep[:].unsqueeze(1).to_broadcast([P, F, l])
        nc.vector.tensor_tensor(out=q3, in0=q3, in1=s_b, op=mybir.AluOpType.mult)
        hh = l // 2
        while hh >= 2:
            nc.vector.tensor_tensor(
                out=q3[:, :, 0:hh],
                in0=q3[:, :, 0:hh],
                in1=q3[:, :, hh : 2 * hh],
                op=mybir.AluOpType.add,
            )
            hh //= 2
        ored = opool.tile([P, F], f32)
        nc.vector.tensor_tensor(
            out=ored[:], in0=q3[:, :, 0], in1=q3[:, :, 1],
            op=mybir.AluOpType.add,
        )
        nc.vector.tensor_scalar_mul(out=ored[:], in0=ored[:], scalar1=rrep[:])

        pending_stores.append(
            (_flat_sample_kc(o_d.ap()[:, i, :], t, c, F), ored[:])
        )
        if len(pending_stores) > 2:
            flush_store()

    while pending_stores:
        flush_store()

    stack.close()


def run(query, key, repeat=1, variant=None, **spmd_kwargs):
    query = np.ascontiguousarray(np.asarray(query, dtype=np.float32))
    key = np.asarray(key, dtype=np.float32)
    n, t, c, l = query.shape
    ncores = NCORES
    ns = n // ncores
    nc = build_nc(ns, t, c, l, variant=variant, repeat=repeat)

    in_maps = []
    for i in range(ncores):
        in_maps.append(
            {
                "q": np.ascontiguousarray(query[i * ns : (i + 1) * ns]),
                "k": np.ascontiguousarray(key[:, i * ns : (i + 1) * ns, :]),
            }
        )
    res = bass_utils.run_bass_kernel_spmd(
        nc, in_maps, core_ids=list(range(ncores)), **spmd_kwargs
    )
    out = np.empty((t, n, c), dtype=np.float32)
    for i in range(ncores):
        out[:, i * ns : (i + 1) * ns, :] = res.results[i]["o"]
    return out, res


def kernel(**inputs):
    out, _ = run(inputs["query"], inputs["key"])
    return out
